# revision 19
# baseline (speedup 1.0000x reference)
"""Multi-head attention (B=8, N=1024, C=768, 12 heads) on 8 TRN2 NeuronCores.

Sharding: data-parallel over batch - batch element b runs on core b, weights
replicated, zero collectives.

v2 design (ScalarE-floor oriented):
  - Inputs shipped bf16 from host: no on-chip weight/x casts, half the DMA.
  - ScalarE runs ONLY the exp stream (96 x [128,1024] activations ~ 110us,
    the hard floor for this shape); activation table preloaded by a dummy
    exp at t=0; first real exp ~14us after DMA lead-in.
  - Scores computed transposed per head (S^T[k,q]), K=64 matmuls. Heads of
    a pair live on partition halves 0-63 / 64-127 so consecutive heads'
    score matmuls land on different PE row groups (tile_position auto) and
    overlap ~2x.
  - P@V per head is a deferred burst during the NEXT head's exp window:
    ev/od split of the key dimension (rows 0-63 / 64-127) -> two concurrent
    row-group matmuls (measured ~2x), qh-split accumulators [65,512] so the
    PSUM acc pool is only 2 banks. v carries a ones column -> row 64 of the
    accumulator is the softmax denominator.
  - normalize: (ev+od) add + reciprocal + gpsimd broadcast + multiply, all
    off the exp critical path.
  - proj pass1 (c-chunks 0..4) runs inside the attention stream into SBUF
    partials with the bias folded in; pass2 (c5) + final add + store is the
    only tail.
  - PSUM: S pool 2x[128,1024] (4 banks) + acc 2x[65,512] (2) + fill
    2x[128,512] (2) = 8 banks exactly.
"""

from contextlib import ExitStack

import numpy as np
import ml_dtypes

import concourse.mybir as mybir
import concourse.tile as tile
from concourse import bacc
from concourse.bass_utils import run_bass_kernel_spmd

B, N, C = 8, 1024, 768
NH, D = 12, 64
CK = C // 128  # 6 contraction chunks of 128
NQ = N // 128  # 8 position chunks of 128
SCALE = D ** -0.5
F32 = mybir.dt.float32
BF16 = mybir.dt.bfloat16
Exp = mybir.ActivationFunctionType.Exp


def _emit(tc, xT, wqkvT, wprojT, bproj, out):
    nc = tc.nc
    with ExitStack() as ctx:
        sb = ctx.enter_context(tc.tile_pool(name="sb", bufs=1))
        pp = ctx.enter_context(tc.tile_pool(name="pp", bufs=24))
        small = ctx.enter_context(tc.tile_pool(name="small", bufs=2))
        spool = ctx.enter_context(tc.tile_pool(name="spool", bufs=2, space="PSUM"))
        accp = ctx.enter_context(tc.tile_pool(name="accp", bufs=2, space="PSUM"))
        fill = ctx.enter_context(tc.tile_pool(name="fill", bufs=2, space="PSUM"))

        # ---- persistent SBUF tensors ---------------------------------
        xT_bf = [sb.tile([128, N], BF16, name=f"xT{c}", tag=f"xT{c}") for c in range(CK)]
        w_bf = [sb.tile([128, 3 * C], BF16, name=f"w{c}", tag=f"w{c}") for c in range(CK)]
        wp_bf = [sb.tile([128, C], BF16, name=f"wp{c}", tag=f"wp{c}") for c in range(CK)]
        qkT = [sb.tile([128, N], BF16, name=f"qkT{m}", tag=f"qkT{m}") for m in range(12)]
        v_sb = [sb.tile([128, NH, D + 1], BF16, name=f"v{n}", tag=f"v{n}") for n in range(NQ)]
        attn_bf = [sb.tile([128, N], BF16, name=f"at{p}", tag=f"at{p}") for p in range(6)]
        y_part = [sb.tile([128, C], BF16, name=f"yp{n}", tag=f"yp{n}") for n in range(NQ)]

        # ---- warm-up + activation table preload ----------------------
        warm_in = sb.tile([128, 512], BF16, name="warm_in", tag="warm_in")
        nc.gpsimd.memset(warm_in[:], 1.0)
        dummy = small.tile([1, 8], F32, name="dummy", tag="dummy")
        nc.scalar.activation(dummy[:], warm_in[0:1, 0:8], Exp, scale=SCALE)
        warm_ps = spool.tile([128, 512], F32, name="warm_ps", tag="s")
        for i in range(24):
            nc.tensor.matmul(
                warm_ps[:],
                lhsT=warm_in[:, 0:128],
                rhs=warm_in[:],
                start=(i == 0),
                stop=(i == 23),
            )

        # ---- DMA schedule --------------------------------------------
        # DMA descriptors serialize ~0.65us each on the Sync engine, so use
        # few, wide transfers. Critical path: per c-chunk, x then w cols
        # 0:896 (covers pair-0 q m=0 AND k m=6, plus q m=1..5 background).
        for c in range(CK):
            nc.sync.dma_start(out=xT_bf[c][:], in_=xT[c * 128:(c + 1) * 128, :])
            nc.sync.dma_start(
                out=w_bf[c][:, 0:896], in_=wqkvT[c * 128:(c + 1) * 128, 0:896]
            )
        # Background: remaining k cols + all v cols in one slice per c.
        for c in range(CK):
            nc.sync.dma_start(
                out=w_bf[c][:, 896:2304], in_=wqkvT[c * 128:(c + 1) * 128, 896:2304]
            )
        for c in range(CK):
            nc.sync.dma_start(out=wp_bf[c][:], in_=wprojT[c * 128:(c + 1) * 128, :])
        bp_row = sb.tile([1, C], F32, name="bp_row", tag="bp_row")
        nc.sync.dma_start(out=bp_row[:], in_=bproj[None, :])
        bias_bc = sb.tile([128, C], F32, name="bias_bc", tag="bias_bc")
        nc.gpsimd.partition_broadcast(bias_bc[:], bp_row[:])

        # ---- qkv building blocks -------------------------------------
        def emit_qk_half(m, qh, pool=fill):
            qk_ps = pool.tile([128, 512], F32, name=f"qk{m}_{qh}", tag="f")
            for c in range(CK):
                nc.tensor.matmul(
                    qk_ps[:],
                    lhsT=w_bf[c][:, m * 128:(m + 1) * 128],
                    rhs=xT_bf[c][:, qh * 512:(qh + 1) * 512],
                    start=(c == 0),
                    stop=(c == CK - 1),
                )
            nc.vector.tensor_copy(qkT[m][:, qh * 512:(qh + 1) * 512], qk_ps[:])

        def emit_v_half(n, half, pool=fill):
            if half == 0:
                nc.gpsimd.memset(v_sb[n][:, :, D], 1.0)
            v_ps = pool.tile([128, 384], F32, name=f"v{n}_{half}", tag="f")
            for c in range(CK):
                nc.tensor.matmul(
                    v_ps[:],
                    lhsT=xT_bf[c][:, n * 128:(n + 1) * 128],
                    rhs=w_bf[c][:, 2 * C + half * 384:2 * C + (half + 1) * 384],
                    start=(c == 0),
                    stop=(c == CK - 1),
                )
            nc.vector.tensor_copy(
                v_sb[n][:, half * 6:(half + 1) * 6, 0:D],
                v_ps[:].rearrange("p (h d) -> p h d", d=D),
            )

        # ---- attention building blocks -------------------------------
        P = {}  # (h, kc) -> SBUF bf16 tile [128, N]

        def emit_S_exp(h, kc):
            """scores S^T[k-chunk, q] + exp -> P[h, kc]."""
            p_idx, ro = h // 2, (h % 2) * 64
            q_tile, k_tile = qkT[p_idx], qkT[6 + p_idx]
            st = spool.tile([128, N], F32, name=f"s{h}_{kc}", tag="s")
            for qh in range(2):
                nc.tensor.matmul(
                    st[:, qh * 512:(qh + 1) * 512],
                    lhsT=k_tile[ro:ro + D, kc * 128:(kc + 1) * 128],
                    rhs=q_tile[ro:ro + D, qh * 512:(qh + 1) * 512],
                    start=True,
                    stop=True,
                )
            pt = pp.tile([128, N], BF16, name=f"P{h}_{kc}", tag="P")
            nc.scalar.activation(pt[:], st[:], Exp, scale=SCALE)
            P[(h, kc)] = pt

        # Per-head PV state: two [65,512] accumulators (qh0/qh1), each fed
        # by 8 K=128 matmuls. (Mixing PE row groups inside one PSUM
        # accumulation group faults the hardware - K=64 split is illegal
        # here, verified empirically.)
        ACC = {}

        def pv_kc(h, kc):
            """one kc step of head h's PV for both query halves."""
            if h not in ACC:
                # head 11 accumulates in the fill pool: its PV overlaps
                # head 10's (which holds accp) during window 5.
                pool = fill if h == 11 else accp
                ACC[h] = [
                    pool.tile([D + 1, 512], F32, name=f"a{h}_0", tag="f" if h == 11 else "a"),
                    pool.tile([D + 1, 512], F32, name=f"a{h}_1", tag="f" if h == 11 else "a"),
                ]
            pt = P[(h, kc)]
            for qh in range(2):
                nc.tensor.matmul(
                    ACC[h][qh][:],
                    lhsT=v_sb[kc][:, h, :],
                    rhs=pt[:, qh * 512:(qh + 1) * 512],
                    start=(kc == 0),
                    stop=(kc == NQ - 1),
                )

        def norm_head(h):
            """reciprocal of the denominator rows + PSUM-direct multiply."""
            p_idx, ro = h // 2, (h % 2) * 64
            acc0, acc1 = ACC.pop(h)
            dn = small.tile([1, N], F32, name=f"dnf{h}", tag="dnf", bufs=2)
            nc.vector.tensor_copy(dn[:, 0:512], acc0[D:D + 1, :])
            nc.vector.tensor_copy(dn[:, 512:1024], acc1[D:D + 1, :])
            rc = small.tile([1, N], F32, name=f"rc{h}", tag="rc", bufs=2)
            nc.vector.reciprocal_approx_fast(rc[:], dn[:])
            bcast = small.tile([D, N], F32, name=f"bc{h}", tag="bc")
            nc.gpsimd.partition_broadcast(bcast[:], rc[:])
            nc.vector.tensor_mul(
                attn_bf[p_idx][ro:ro + D, 0:512], acc0[0:D, :], bcast[:, 0:512]
            )
            nc.vector.tensor_mul(
                attn_bf[p_idx][ro:ro + D, 512:1024], acc1[0:D, :], bcast[:, 512:1024]
            )
            for kc in range(NQ):
                del P[(h, kc)]

        def pv_closures(h):
            """per-kc closures (fine-grained for PE interleaving) + norm."""
            cls = [lambda h=h, kc=kc: pv_kc(h, kc) for kc in range(NQ)]
            cls.append(lambda h=h: norm_head(h))
            return cls

        # ---- proj building blocks ------------------------------------
        def proj_pass1(n):
            """c-chunks 0..3 of the projection for n-chunk n, bias folded,
            into y_part[n]. lo/hi column phases keep the PSUM use 1-bank."""
            for lo, hi in ((0, 512), (512, 768)):
                y_ps = fill.tile([128, hi - lo], F32, name=f"y1_{n}_{lo}", tag="f")
                for c in range(4):
                    nc.tensor.matmul(
                        y_ps[:],
                        lhsT=attn_bf[c][:, n * 128:(n + 1) * 128],
                        rhs=wp_bf[c][:, lo:hi],
                        start=(c == 0),
                        stop=(c == 3),
                    )
                nc.vector.tensor_add(y_part[n][:, lo:hi], y_ps[:], bias_bc[:, lo:hi])

        def proj_pass2(n, pool=fill):
            for lo, hi in ((0, 512), (512, 768)):
                y_ps = pool.tile([128, hi - lo], F32, name=f"y2_{n}_{lo}", tag="f")
                for c in (4, 5):
                    nc.tensor.matmul(
                        y_ps[:],
                        lhsT=attn_bf[c][:, n * 128:(n + 1) * 128],
                        rhs=wp_bf[c][:, lo:hi],
                        start=(c == 4),
                        stop=(c == 5),
                    )
                y_sb = small.tile([128, hi - lo], F32, name=f"ys{n}_{lo}", tag="ys", bufs=2)
                nc.vector.tensor_add(y_sb[:], y_ps[:], y_part[n][:, lo:hi])
                nc.sync.dma_start(
                    out=out[n * 128:(n + 1) * 128, lo:hi], in_=y_sb[:]
                )

        # ---- lead-in compute -----------------------------------------
        emit_qk_half(0, 0)
        emit_qk_half(0, 1)
        emit_qk_half(6, 0)
        emit_qk_half(6, 1)
        emit_v_half(0, 0)
        emit_v_half(0, 1)

        # ---- attention stream ----------------------------------------
        # Pair p covers heads A=2p, B=2p+1 sharing qkT tiles p / 6+p.
        # Exp order is kc-major (A0,B0,A1,B1,...) so consecutive score
        # matmuls alternate PE row groups; the K=64 PV steps of the
        # previous pair ride the complementary row group. Filler closures
        # drain evenly across the 16 exp slots of each window; pv(A) goes
        # early and pv(B) late with other work between, so the accp pool
        # handoff (norm latency) hides behind real work.
        def qk_halves(*ms):
            return [
                lambda m=m, qh=qh: emit_qk_half(m, qh)
                for m in ms
                for qh in range(2)
            ]

        def window_fillers(p):
            if p == 0:
                return qk_halves(1, 7) + [
                    lambda n=n, h=half: emit_v_half(n, h)
                    for n in range(1, NQ)
                    for half in range(2)
                ]
            f = pv_closures(2 * (p - 1))
            mid = []
            if p == 1:
                mid += qk_halves(2, 8)
            elif p <= 4:
                mid += qk_halves(p + 1, 6 + p + 1)
            f += mid
            f += pv_closures(2 * (p - 1) + 1)
            if p == 4:
                # needs pair 3 (heads 6,7 -> attn_bf[3]) normalized above
                f += [lambda n=n: proj_pass1(n) for n in range(2)]
            if p == 5:
                # pass1 first (it uses the fill pool, which head 11's
                # incremental PV takes over mid-window), then pair-4 PV.
                f = [lambda n=n: proj_pass1(n) for n in range(2, NQ)] + f
                f += pv_closures(10)
            return f

        for p in range(6):
            fillers = window_fillers(p)
            n_slots = 16
            done = 0
            slot = 0
            # window 5 runs A-major so head 10's P tiles complete by slot
            # 8 and its PV can drain in-window; earlier windows kc-major.
            if p == 5:
                order = [(h, kc) for h in (10, 11) for kc in range(NQ)]
            else:
                order = [(2 * p + i, kc) for kc in range(NQ) for i in (0, 1)]
            for h, kc in order:
                emit_S_exp(h, kc)
                if h == 11:
                    # incremental PV for the last head - kills the tail
                    pv_kc(11, kc)
                slot += 1
                want = (slot * len(fillers)) // n_slots
                while done < want:
                    fillers[done]()
                    done += 1
            while done < len(fillers):
                fillers[done]()
                done += 1

        # ---- tail ----------------------------------------------------
        norm_head(11)
        for n in range(NQ):
            proj_pass2(n)


def build_graph():
    nc = bacc.Bacc("TRN2", target_bir_lowering=False, debug=False)
    xT = nc.declare_dram_parameter("xT", [C, N], BF16, isOutput=False)
    wqkvT = nc.declare_dram_parameter("wqkvT", [C, 3 * C], BF16, isOutput=False)
    wprojT = nc.declare_dram_parameter("wprojT", [C, C], BF16, isOutput=False)
    bproj = nc.declare_dram_parameter("bproj", [C], F32, isOutput=False)
    out = nc.declare_dram_parameter("out", [N, C], F32, isOutput=True)
    with tile.TileContext(nc) as tc:
        _emit(tc, xT.ap(), wqkvT.ap(), wprojT.ap(), bproj.ap(), out.ap())
    nc.compile()
    return nc


_GRAPH = None


def _get_graph():
    global _GRAPH
    if _GRAPH is None:
        _GRAPH = build_graph()
    return _GRAPH


def make_in_maps(x, W_qkv, W_proj, b_proj):
    x = np.asarray(x, dtype=np.float32)
    wqkvT = np.ascontiguousarray(
        np.asarray(W_qkv, dtype=np.float32).T.astype(ml_dtypes.bfloat16)
    )
    wprojT = np.ascontiguousarray(
        np.asarray(W_proj, dtype=np.float32).T.astype(ml_dtypes.bfloat16)
    )
    bp = np.ascontiguousarray(np.asarray(b_proj, dtype=np.float32))
    xT_all = np.ascontiguousarray(
        x.transpose(0, 2, 1).astype(ml_dtypes.bfloat16)
    )
    return [
        {"xT": xT_all[i], "wqkvT": wqkvT, "wprojT": wprojT, "bproj": bp}
        for i in range(B)
    ]


def run(x, W_qkv, W_proj, b_proj, trace=False):
    nc = _get_graph()
    in_maps = make_in_maps(x, W_qkv, W_proj, b_proj)
    res = run_bass_kernel_spmd(nc, in_maps, core_ids=list(range(B)), trace=trace)
    out = np.stack([res.results[i]["out"] for i in range(B)], axis=0)
    return out.astype(np.float32, copy=False), res


def kernel(x, W_qkv, W_proj, b_proj, H=None, W=None):
    out, _ = run(x, W_qkv, W_proj, b_proj)
    return out


# revision 20
# speedup vs baseline: 1.1142x; 1.1142x over previous
"""Multi-head attention (B=8, N=1024, C=768, 12 heads) on 8 TRN2 NeuronCores.

Sharding: data-parallel over batch - batch element b runs on core b, weights
replicated, zero collectives.

v2 design (ScalarE-floor oriented):
  - Inputs shipped bf16 from host: no on-chip weight/x casts, half the DMA.
  - ScalarE runs ONLY the exp stream (96 x [128,1024] activations ~ 110us,
    the hard floor for this shape); activation table preloaded by a dummy
    exp at t=0; first real exp ~14us after DMA lead-in.
  - Scores computed transposed per head (S^T[k,q]), K=64 matmuls. Heads of
    a pair live on partition halves 0-63 / 64-127 so consecutive heads'
    score matmuls land on different PE row groups (tile_position auto) and
    overlap ~2x.
  - P@V per head is a deferred burst during the NEXT head's exp window:
    ev/od split of the key dimension (rows 0-63 / 64-127) -> two concurrent
    row-group matmuls (measured ~2x), qh-split accumulators [65,512] so the
    PSUM acc pool is only 2 banks. v carries a ones column -> row 64 of the
    accumulator is the softmax denominator.
  - normalize: (ev+od) add + reciprocal + gpsimd broadcast + multiply, all
    off the exp critical path.
  - proj pass1 (c-chunks 0..4) runs inside the attention stream into SBUF
    partials with the bias folded in; pass2 (c5) + final add + store is the
    only tail.
  - PSUM: S pool 2x[128,1024] (4 banks) + acc 2x[65,512] (2) + fill
    2x[128,512] (2) = 8 banks exactly.
"""

from contextlib import ExitStack

import numpy as np
import ml_dtypes

import concourse.mybir as mybir
import concourse.tile as tile
from concourse import bacc
from concourse.bass_utils import run_bass_kernel_spmd

B, N, C = 8, 1024, 768
NH, D = 12, 64
CK = C // 128  # 6 contraction chunks of 128
NQ = N // 128  # 8 position chunks of 128
SCALE = D ** -0.5
F32 = mybir.dt.float32
BF16 = mybir.dt.bfloat16
Exp = mybir.ActivationFunctionType.Exp


def _emit(tc, xT, wqkvT, wprojT, bproj, out):
    nc = tc.nc
    with ExitStack() as ctx:
        sb = ctx.enter_context(tc.tile_pool(name="sb", bufs=1))
        pp = ctx.enter_context(tc.tile_pool(name="pp", bufs=24))
        small = ctx.enter_context(tc.tile_pool(name="small", bufs=2))
        spool = ctx.enter_context(tc.tile_pool(name="spool", bufs=2, space="PSUM"))
        accp = ctx.enter_context(tc.tile_pool(name="accp", bufs=2, space="PSUM"))
        fill = ctx.enter_context(tc.tile_pool(name="fill", bufs=2, space="PSUM"))

        # ---- persistent SBUF tensors ---------------------------------
        xT_bf = [sb.tile([128, N], BF16, name=f"xT{c}", tag=f"xT{c}") for c in range(CK)]
        w_bf = [sb.tile([128, 3 * C], BF16, name=f"w{c}", tag=f"w{c}") for c in range(CK)]
        wp_bf = [sb.tile([128, C], BF16, name=f"wp{c}", tag=f"wp{c}") for c in range(CK)]
        qkT = [sb.tile([128, N], BF16, name=f"qkT{m}", tag=f"qkT{m}") for m in range(12)]
        v_sb = [sb.tile([128, NH, D + 1], BF16, name=f"v{n}", tag=f"v{n}") for n in range(NQ)]
        attn_bf = [sb.tile([128, N], BF16, name=f"at{p}", tag=f"at{p}") for p in range(6)]
        y_part = [sb.tile([128, C], BF16, name=f"yp{n}", tag=f"yp{n}") for n in range(NQ)]

        # ---- warm-up + activation table preload ----------------------
        warm_in = sb.tile([128, 512], BF16, name="warm_in", tag="warm_in")
        nc.gpsimd.memset(warm_in[:], 1.0)
        dummy = small.tile([1, 8], F32, name="dummy", tag="dummy")
        nc.scalar.activation(dummy[:], warm_in[0:1, 0:8], Exp, scale=SCALE)
        warm_ps = spool.tile([128, 512], F32, name="warm_ps", tag="s")
        for i in range(24):
            nc.tensor.matmul(
                warm_ps[:],
                lhsT=warm_in[:, 0:128],
                rhs=warm_in[:],
                start=(i == 0),
                stop=(i == 23),
            )

        # ---- DMA schedule --------------------------------------------
        # DMA descriptors serialize ~0.65us each on the Sync engine, so use
        # few, wide transfers. Critical path: per c-chunk, x then w cols
        # 0:896 (covers pair-0 q m=0 AND k m=6, plus q m=1..5 background).
        for c in range(CK):
            nc.sync.dma_start(out=xT_bf[c][:], in_=xT[c * 128:(c + 1) * 128, :])
            nc.sync.dma_start(
                out=w_bf[c][:, 0:896], in_=wqkvT[c * 128:(c + 1) * 128, 0:896]
            )
        # Background: remaining k cols + all v cols in one slice per c.
        for c in range(CK):
            nc.sync.dma_start(
                out=w_bf[c][:, 896:2304], in_=wqkvT[c * 128:(c + 1) * 128, 896:2304]
            )
        for c in range(CK):
            nc.sync.dma_start(out=wp_bf[c][:], in_=wprojT[c * 128:(c + 1) * 128, :])
        bp_row = sb.tile([1, C], F32, name="bp_row", tag="bp_row")
        nc.sync.dma_start(out=bp_row[:], in_=bproj[None, :])
        bias_bc = sb.tile([128, C], F32, name="bias_bc", tag="bias_bc")
        nc.gpsimd.partition_broadcast(bias_bc[:], bp_row[:])

        # ---- qkv building blocks -------------------------------------
        def emit_qk_half(m, qh, pool=fill):
            qk_ps = pool.tile([128, 512], F32, name=f"qk{m}_{qh}", tag="f")
            for c in range(CK):
                nc.tensor.matmul(
                    qk_ps[:],
                    lhsT=w_bf[c][:, m * 128:(m + 1) * 128],
                    rhs=xT_bf[c][:, qh * 512:(qh + 1) * 512],
                    start=(c == 0),
                    stop=(c == CK - 1),
                )
            nc.vector.tensor_copy(qkT[m][:, qh * 512:(qh + 1) * 512], qk_ps[:])

        def emit_v_half(n, half, pool=fill):
            if half == 0:
                nc.gpsimd.memset(v_sb[n][:, :, D], 1.0)
            v_ps = pool.tile([128, 384], F32, name=f"v{n}_{half}", tag="f")
            for c in range(CK):
                nc.tensor.matmul(
                    v_ps[:],
                    lhsT=xT_bf[c][:, n * 128:(n + 1) * 128],
                    rhs=w_bf[c][:, 2 * C + half * 384:2 * C + (half + 1) * 384],
                    start=(c == 0),
                    stop=(c == CK - 1),
                )
            nc.vector.tensor_copy(
                v_sb[n][:, half * 6:(half + 1) * 6, 0:D],
                v_ps[:].rearrange("p (h d) -> p h d", d=D),
            )

        # ---- attention building blocks -------------------------------
        P = {}  # (h, kc) -> SBUF bf16 tile [128, N]

        def emit_S_exp(h, kc):
            """scores S^T[k-chunk, q] + exp -> P[h, kc]."""
            p_idx, ro = h // 2, (h % 2) * 64
            q_tile, k_tile = qkT[p_idx], qkT[6 + p_idx]
            st = spool.tile([128, N], F32, name=f"s{h}_{kc}", tag="s")
            for qh in range(2):
                nc.tensor.matmul(
                    st[:, qh * 512:(qh + 1) * 512],
                    lhsT=k_tile[ro:ro + D, kc * 128:(kc + 1) * 128],
                    rhs=q_tile[ro:ro + D, qh * 512:(qh + 1) * 512],
                    start=True,
                    stop=True,
                )
            pt = pp.tile([128, N], BF16, name=f"P{h}_{kc}", tag="P")
            nc.scalar.activation(pt[:], st[:], Exp, scale=SCALE)
            P[(h, kc)] = pt

        # Per-head PV state: two [65,512] accumulators (qh0/qh1), each fed
        # by 8 K=128 matmuls. (Mixing PE row groups inside one PSUM
        # accumulation group faults the hardware - K=64 split is illegal
        # here, verified empirically.)
        ACC = {}

        def pv_kc(h, kc):
            """one kc step of head h's PV for both query halves."""
            if h not in ACC:
                # head 11 accumulates in the fill pool: its PV overlaps
                # head 10's (which holds accp) during window 5.
                pool = fill if h == 11 else accp
                ACC[h] = [
                    pool.tile([D + 1, 512], F32, name=f"a{h}_0", tag="f" if h == 11 else "a"),
                    pool.tile([D + 1, 512], F32, name=f"a{h}_1", tag="f" if h == 11 else "a"),
                ]
            pt = P[(h, kc)]
            for qh in range(2):
                nc.tensor.matmul(
                    ACC[h][qh][:],
                    lhsT=v_sb[kc][:, h, :],
                    rhs=pt[:, qh * 512:(qh + 1) * 512],
                    start=(kc == 0),
                    stop=(kc == NQ - 1),
                )

        def norm_head(h):
            """reciprocal of the denominator rows + PSUM-direct multiply."""
            p_idx, ro = h // 2, (h % 2) * 64
            acc0, acc1 = ACC.pop(h)
            dn = small.tile([1, N], F32, name=f"dnf{h}", tag="dnf", bufs=2)
            nc.vector.tensor_copy(dn[:, 0:512], acc0[D:D + 1, :])
            nc.vector.tensor_copy(dn[:, 512:1024], acc1[D:D + 1, :])
            rc = small.tile([1, N], F32, name=f"rc{h}", tag="rc", bufs=2)
            nc.vector.reciprocal_approx_fast(rc[:], dn[:])
            bcast = small.tile([D, N], F32, name=f"bc{h}", tag="bc")
            nc.gpsimd.partition_broadcast(bcast[:], rc[:])
            nc.vector.tensor_mul(
                attn_bf[p_idx][ro:ro + D, 0:512], acc0[0:D, :], bcast[:, 0:512]
            )
            nc.vector.tensor_mul(
                attn_bf[p_idx][ro:ro + D, 512:1024], acc1[0:D, :], bcast[:, 512:1024]
            )
            for kc in range(NQ):
                del P[(h, kc)]

        def pv_closures(h):
            """per-kc closures (fine-grained for PE interleaving) + norm."""
            cls = [lambda h=h, kc=kc: pv_kc(h, kc) for kc in range(NQ)]
            cls.append(lambda h=h: norm_head(h))
            return cls

        # ---- proj building blocks ------------------------------------
        def proj_pass1(n):
            """c-chunks 0..3 of the projection for n-chunk n, bias folded,
            into y_part[n]. lo/hi column phases keep the PSUM use 1-bank."""
            for lo, hi in ((0, 512), (512, 768)):
                y_ps = fill.tile([128, hi - lo], F32, name=f"y1_{n}_{lo}", tag="f")
                for c in range(4):
                    nc.tensor.matmul(
                        y_ps[:],
                        lhsT=attn_bf[c][:, n * 128:(n + 1) * 128],
                        rhs=wp_bf[c][:, lo:hi],
                        start=(c == 0),
                        stop=(c == 3),
                    )
                nc.vector.tensor_add(y_part[n][:, lo:hi], y_ps[:], bias_bc[:, lo:hi])

        def proj_pass2(n, pool=fill):
            for lo, hi in ((0, 512), (512, 768)):
                y_ps = pool.tile([128, hi - lo], F32, name=f"y2_{n}_{lo}", tag="f")
                for c in (4, 5):
                    nc.tensor.matmul(
                        y_ps[:],
                        lhsT=attn_bf[c][:, n * 128:(n + 1) * 128],
                        rhs=wp_bf[c][:, lo:hi],
                        start=(c == 4),
                        stop=(c == 5),
                    )
                y_sb = small.tile([128, hi - lo], F32, name=f"ys{n}_{lo}", tag="ys", bufs=2)
                nc.vector.tensor_add(y_sb[:], y_ps[:], y_part[n][:, lo:hi])
                nc.sync.dma_start(
                    out=out[n * 128:(n + 1) * 128, lo:hi], in_=y_sb[:]
                )

        # ---- lead-in compute -----------------------------------------
        emit_qk_half(0, 0)
        emit_qk_half(0, 1)
        emit_qk_half(6, 0)
        emit_qk_half(6, 1)
        emit_v_half(0, 0)
        emit_v_half(0, 1)

        # ---- attention stream ----------------------------------------
        # Pair p covers heads A=2p, B=2p+1 sharing qkT tiles p / 6+p.
        # Exp order is kc-major (A0,B0,A1,B1,...) so consecutive score
        # matmuls alternate PE row groups; the K=64 PV steps of the
        # previous pair ride the complementary row group. Filler closures
        # drain evenly across the 16 exp slots of each window; pv(A) goes
        # early and pv(B) late with other work between, so the accp pool
        # handoff (norm latency) hides behind real work.
        def qk_halves(*ms):
            return [
                lambda m=m, qh=qh: emit_qk_half(m, qh)
                for m in ms
                for qh in range(2)
            ]

        def window_fillers(p):
            if p == 0:
                return qk_halves(1, 7) + [
                    lambda n=n, h=half: emit_v_half(n, h)
                    for n in range(1, NQ)
                    for half in range(2)
                ]
            f = pv_closures(2 * (p - 1))
            mid = []
            if p == 1:
                mid += qk_halves(2, 8)
            elif p == 2:
                mid += qk_halves(3, 9) + qk_halves(4, 10)
            elif p == 3:
                mid += qk_halves(5, 11)
            f += mid
            f += pv_closures(2 * (p - 1) + 1)
            if p == 4:
                # needs pair 3 (heads 6,7 -> attn_bf[3]) normalized above
                f += [lambda n=n: proj_pass1(n) for n in range(6)]
            if p == 5:
                # pass1 leftovers first (they use the fill pool, which head
                # 11's incremental PV takes over mid-window), then pv(10).
                f = [lambda n=n: proj_pass1(n) for n in range(6, NQ)] + f
                f += pv_closures(10)
            return f

        for p in range(6):
            fillers = window_fillers(p)
            n_slots = 16
            done = 0
            slot = 0
            # window 5 runs A-major so head 10's P tiles complete by slot
            # 8 and its PV can drain in-window; earlier windows kc-major.
            if p == 5:
                order = [(h, kc) for h in (10, 11) for kc in range(NQ)]
            else:
                order = [(2 * p + i, kc) for kc in range(NQ) for i in (0, 1)]
            for h, kc in order:
                emit_S_exp(h, kc)
                if h == 11:
                    # incremental PV for the last head - kills the tail
                    pv_kc(11, kc)
                slot += 1
                want = (slot * len(fillers)) // n_slots
                while done < want:
                    fillers[done]()
                    done += 1
            while done < len(fillers):
                fillers[done]()
                done += 1

        # ---- tail ----------------------------------------------------
        norm_head(11)
        for n in range(NQ):
            proj_pass2(n)


def build_graph():
    nc = bacc.Bacc("TRN2", target_bir_lowering=False, debug=False)
    xT = nc.declare_dram_parameter("xT", [C, N], BF16, isOutput=False)
    wqkvT = nc.declare_dram_parameter("wqkvT", [C, 3 * C], BF16, isOutput=False)
    wprojT = nc.declare_dram_parameter("wprojT", [C, C], BF16, isOutput=False)
    bproj = nc.declare_dram_parameter("bproj", [C], F32, isOutput=False)
    out = nc.declare_dram_parameter("out", [N, C], F32, isOutput=True)
    with tile.TileContext(nc) as tc:
        _emit(tc, xT.ap(), wqkvT.ap(), wprojT.ap(), bproj.ap(), out.ap())
    nc.compile()
    return nc


_GRAPH = None


def _get_graph():
    global _GRAPH
    if _GRAPH is None:
        _GRAPH = build_graph()
    return _GRAPH


def make_in_maps(x, W_qkv, W_proj, b_proj):
    x = np.asarray(x, dtype=np.float32)
    wqkvT = np.ascontiguousarray(
        np.asarray(W_qkv, dtype=np.float32).T.astype(ml_dtypes.bfloat16)
    )
    wprojT = np.ascontiguousarray(
        np.asarray(W_proj, dtype=np.float32).T.astype(ml_dtypes.bfloat16)
    )
    bp = np.ascontiguousarray(np.asarray(b_proj, dtype=np.float32))
    xT_all = np.ascontiguousarray(
        x.transpose(0, 2, 1).astype(ml_dtypes.bfloat16)
    )
    return [
        {"xT": xT_all[i], "wqkvT": wqkvT, "wprojT": wprojT, "bproj": bp}
        for i in range(B)
    ]


def run(x, W_qkv, W_proj, b_proj, trace=False):
    nc = _get_graph()
    in_maps = make_in_maps(x, W_qkv, W_proj, b_proj)
    res = run_bass_kernel_spmd(nc, in_maps, core_ids=list(range(B)), trace=trace)
    out = np.stack([res.results[i]["out"] for i in range(B)], axis=0)
    return out.astype(np.float32, copy=False), res


def kernel(x, W_qkv, W_proj, b_proj, H=None, W=None):
    out, _ = run(x, W_qkv, W_proj, b_proj)
    return out


# revision 26
# speedup vs baseline: 1.3319x; 1.1954x over previous
"""Multi-head attention (B=8, N=1024, C=768, 12 heads) on 8 TRN2 NeuronCores.

Sharding: data-parallel over batch — batch element b runs on core b, weights
replicated, zero collectives.

Per-core kernel (all matmuls bf16 on the TensorEngine):
  - Host pre-transposes x, W_qkv, W_proj so every contraction has its
    reduction axis on SBUF partitions; no on-device transposes needed.
  - qkv: q^T,k^T [768,1024] and v [1024,768] via 6-chunk K=768 matmuls.
  - scores are computed TRANSPOSED per head: S^T[k,q] with lhsT=k^T-block,
    rhs=q^T-block, so the exp output P^T feeds the P@V matmul directly as
    the moving operand (no transpose of the attention matrix). The
    1/sqrt(d) scale rides for free on the exp's affine pre-scale.
  - softmax denominators come free: v is stored with a ones-column
    appended per head (lhsT [128,65]); row 64 of the P@V accumulator is
    sum_k exp(S), i.e. the denominator.
  - normalization runs entirely off the TensorEngine's critical path:
    copy the accumulator to SBUF (releasing its PSUM slot), fast
    approximate reciprocal on VectorE, broadcast across partitions on the
    (otherwise idle) GpSimd engine, one elementwise multiply per head.
  - proj: y = attn @ W_proj^T + b_proj, bias materialized once via
    partition_broadcast and added during the PSUM->SBUF staging.
  - qkv chunk emission is interleaved into the attention stream as
    filler so the TensorEngine stays dense while ScalarE works through
    the exps.
"""

from contextlib import ExitStack

import numpy as np

import concourse.mybir as mybir
import concourse.tile as tile
from concourse import bacc
from concourse.bass_utils import run_bass_kernel_spmd

B, N, C = 8, 1024, 768
NH, D = 12, 64
CK = C // 128  # 6 contraction chunks of 128
NQ = N // 128  # 8 position chunks of 128
SCALE = D ** -0.5
F32 = mybir.dt.float32
BF16 = mybir.dt.bfloat16
Copy = mybir.ActivationFunctionType.Copy
Exp = mybir.ActivationFunctionType.Exp


def _emit(tc, xT, wqkvT, wprojT, bproj, out):
    nc = tc.nc
    with ExitStack() as ctx:
        sb = ctx.enter_context(tc.tile_pool(name="sb", bufs=1))
        stage = ctx.enter_context(tc.tile_pool(name="stage", bufs=12))
        pp = ctx.enter_context(tc.tile_pool(name="pp", bufs=8))
        small = ctx.enter_context(tc.tile_pool(name="small", bufs=2))
        # PSUM pools are released by hand: qkv+attention use ps/acc, the
        # projection reuses the freed banks for a deeper y pipeline.
        ps = tc.alloc_tile_pool(name="ps", bufs=3, space="PSUM")
        acc = tc.alloc_tile_pool(name="acc", bufs=1, space="PSUM")

        # ---- PE warm-up ----------------------------------------------
        # The TensorEngine is idle through the initial DMA lead-in; HAM
        # then starts the first real matmuls at half clock. Keep the PE
        # busy on scratch work so it enters the qkv phase warm.
        warm_in = sb.tile([128, 512], BF16, name="warm_in", tag="warm_in")
        nc.gpsimd.memset(warm_in[:], 1.0)
        warm_ps = ps.tile([128, 512], F32, name="warm_ps", tag="s")
        for i in range(24):
            nc.tensor.matmul(
                warm_ps[:],
                lhsT=warm_in[:, 0:128],
                rhs=warm_in[:],
                start=(i == 0),
                stop=(i == 23),
            )

        # ---- load + bf16-convert x^T and W_qkv^T ---------------------
        # The first scores matmul needs all of x^T plus the q-left and
        # k-left weight columns, so those loads are interleaved per
        # c-chunk; v and the right halves follow.
        xT_bf = [
            sb.tile([128, N], BF16, name=f"xT_bf{c}", tag=f"xT_bf{c}")
            for c in range(CK)
        ]
        wq_bf = [
            sb.tile([128, 3 * C], BF16, name=f"wq_bf{c}", tag=f"wq_bf{c}")
            for c in range(CK)
        ]

        # Inputs arrive bf16 from the host: plain DMAs, no staging, no
        # casts, half the bytes. Critical path per c-chunk: x, then w cols
        # 0:1152 (q all + k-left). Everything else streams in behind.
        for c in range(CK):
            nc.sync.dma_start(
                out=xT_bf[c][:], in_=xT[c * 128:(c + 1) * 128, :]
            )
            nc.sync.dma_start(
                out=wq_bf[c][:, 0:1152], in_=wqkvT[c * 128:(c + 1) * 128, 0:1152]
            )
        for c in range(CK):
            nc.sync.dma_start(
                out=wq_bf[c][:, 1152:2304],
                in_=wqkvT[c * 128:(c + 1) * 128, 1152:2304],
            )

        # ---- qkv projections -----------------------------------------
        # q^T,k^T: chunk m covers rows [m*128,(m+1)*128) of qkv^T;
        # m in 0..5 -> q, m in 6..11 -> k.
        qkT = [
            sb.tile([128, N], BF16, name=f"qkT{m}", tag=f"qkT{m}")
            for m in range(12)
        ]

        def emit_qk(m):
            for qh in range(2):
                qk_ps = ps.tile([128, 512], F32, name=f"qk_ps{m}_{qh}", tag="s")
                for c in range(CK):
                    nc.tensor.matmul(
                        qk_ps[:],
                        lhsT=wq_bf[c][:, m * 128:(m + 1) * 128],
                        rhs=xT_bf[c][:, qh * 512:(qh + 1) * 512],
                        start=(c == 0),
                        stop=(c == CK - 1),
                    )
                nc.vector.tensor_copy(qkT[m][:, qh * 512:(qh + 1) * 512], qk_ps[:])

        # v in natural layout [n, (head, d)] with a ones column appended
        # per head: v_sb[n] is [128, NH, D+1], [:, h, D] == 1.0.
        v_sb = [
            sb.tile([128, NH, D + 1], BF16, name=f"v_sb{n}", tag=f"v_sb{n}")
            for n in range(NQ)
        ]

        def emit_v(n):
            nc.gpsimd.memset(v_sb[n][:, :, D], 1.0)
            for half in range(2):
                v_ps = ps.tile([128, 384], F32, name=f"v_ps{n}_{half}", tag="s")
                for c in range(CK):
                    nc.tensor.matmul(
                        v_ps[:],
                        lhsT=xT_bf[c][:, n * 128:(n + 1) * 128],
                        rhs=wq_bf[c][:, 2 * C + half * 384:2 * C + (half + 1) * 384],
                        start=(c == 0),
                        stop=(c == CK - 1),
                    )
                nc.vector.tensor_copy(
                    v_sb[n][:, half * 6:(half + 1) * 6, 0:D],
                    v_ps[:].rearrange("p (h d) -> p h d", d=D),
                )

        # ---- attention ------------------------------------------------
        attn_bf = [
            sb.tile([128, N], BF16, name=f"attn_bf{p}", tag=f"attn_bf{p}")
            for p in range(6)
        ]

        def emit_head(h, filler=None):
            """S^T + exp + P@V for head h; `filler` emits extra PE work
            early in the stream (previous head's deferred normalize, next
            qkv chunk) so PE has exp-independent work while ScalarE runs."""
            q_tile = qkT[h // 2]
            k_tile = qkT[6 + h // 2]
            ro = (h % 2) * 64
            out_aug = acc.tile([D + 1, N], F32, name=f"oaug{h}", tag="acc")

            def emit_S(kc):
                st = ps.tile([128, N], F32, name=f"s{h}_{kc}", tag="s")
                for qh in range(2):
                    nc.tensor.matmul(
                        st[:, qh * 512:(qh + 1) * 512],
                        lhsT=k_tile[ro:ro + D, kc * 128:(kc + 1) * 128],
                        rhs=q_tile[ro:ro + D, qh * 512:(qh + 1) * 512],
                        start=True,
                        stop=True,
                    )
                pt = pp.tile([128, N], BF16, name=f"P{h}_{kc}", tag="P")
                nc.scalar.activation(pt[:], st[:], Exp, scale=SCALE)
                return pt

            def emit_v_mm(kc, pt):
                for qh in range(2):
                    nc.tensor.matmul(
                        out_aug[:, qh * 512:(qh + 1) * 512],
                        lhsT=v_sb[kc][:, h, :],
                        rhs=pt[:, qh * 512:(qh + 1) * 512],
                        start=(kc == 0),
                        stop=(kc == NQ - 1),
                    )

            # software pipeline: exp(kc) overlaps S(kc+1) and P@V(kc-1)
            pts = {0: emit_S(0), 1: emit_S(1)}
            if filler is not None:
                filler()
            for kc in range(NQ):
                emit_v_mm(kc, pts.pop(kc))
                if kc + 2 < NQ:
                    pts[kc + 2] = emit_S(kc + 2)
            return out_aug

        def emit_norm_pre(h, oa, direct=False):
            """DVE/GpSimd-only part: reciprocal chain first (it gates the
            final multiply), then stage the accumulator to SBUF to release
            its PSUM slot. For the last head (`direct`) the multiply reads
            the accumulator straight from PSUM instead — shortest tail."""
            if not direct:
                # staging copy FIRST: it releases the single-slot PSUM
                # accumulator, which gates the next head's P@V matmuls
                un = small.tile([D, N], F32, name=f"un{h}", tag="un")
                nc.vector.tensor_copy(un[:], oa[0:D, :])
            dn = small.tile([1, N], F32, name=f"dn{h}", tag="dn")
            nc.vector.tensor_copy(dn[:], oa[D:D + 1, :])
            rc = small.tile([1, N], F32, name=f"rc{h}", tag="rc")
            # reciprocal_approx_fast's uOp program only works from
            # partition 0 on hardware, hence the dn bounce copy above.
            nc.vector.reciprocal_approx_fast(rc[:], dn[:])
            rcb = small.tile([1, N], BF16, name=f"rcb{h}", tag="rcb")
            nc.vector.tensor_copy(rcb[:], rc[:])
            bcast = small.tile([64, N], BF16, name=f"bcast{h}", tag="bcast")
            nc.gpsimd.partition_broadcast(bcast[:], rcb[:])
            if direct:
                return oa, bcast
            return un, bcast

        def emit_norm_post(h, un, bcast):
            p, ro = h // 2, (h % 2) * 64
            nc.vector.tensor_mul(attn_bf[p][ro:ro + 64, :], un[0:D, :], bcast[:])

        # proj weights + bias: pure DMAs now, issue behind the qkv loads
        # so the projection is never gated on them later.
        wp_bf = [
            sb.tile([128, C], BF16, name=f"wp_bf{c}", tag=f"wp_bf{c}")
            for c in range(CK)
        ]
        for c in range(CK):
            nc.sync.dma_start(
                out=wp_bf[c][:], in_=wprojT[c * 128:(c + 1) * 128, :]
            )
        bp_row = sb.tile([1, C], F32, name="bp_row", tag="bp_row")
        nc.sync.dma_start(out=bp_row[:], in_=bproj[None, :])
        bias_bc = sb.tile([128, C], F32, name="bias_bc", tag="bias_bc")
        nc.gpsimd.partition_broadcast(bias_bc[:], bp_row[:])

        emit_qk(0)
        emit_qk(6)
        emit_v(0)

        # Remaining qkv work rides inside the attention stream as PE
        # filler during exp waits: head 0 carries the other v chunks
        # (needed from its own P@V loop onward); later heads each carry
        # one q/k chunk, landing one pair ahead of first use.
        QK_FILL = {1: (1, 7), 2: (2,), 3: (8,), 4: (3,), 5: (9,),
                   6: (4,), 7: (10,), 8: (5,), 9: (11,)}

        pending = None
        for h in range(NH):
            fillers = []
            if h == 0:
                fillers.append(lambda: [emit_v(n) for n in range(1, NQ)])
            for m in QK_FILL.get(h, ()):
                fillers.append(lambda m=m: emit_qk(m))
            if pending is not None:
                ph, un, bc = pending
                fillers.append(lambda ph=ph, un=un, bc=bc: emit_norm_post(ph, un, bc))

            def filler():
                for f in fillers:
                    f()

            oa = emit_head(h, filler=filler)
            pending = (h, *emit_norm_pre(h, oa))
        emit_norm_post(*pending)

        # ---- output projection ---------------------------------------
        acc.release()
        ps.release()
        yps = tc.alloc_tile_pool(name="yps", bufs=3, space="PSUM")

        # Groups of 3 n-chunks, two sweeps each: the c<5 accumulations of
        # a whole group run first (PE work that doesn't need attn_bf[5],
        # absorbing head 11's normalize-chain latency), then the c=5
        # closers + bias-add + store.
        for grp in (range(0, 3), range(3, 6), range(6, NQ)):
            y_tiles = {}
            for n in grp:
                y_ps = yps.tile([128, C], F32, name=f"y_ps{n}", tag="y_ps")
                y_tiles[n] = y_ps
                for lo, hi in ((0, 512), (512, 768)):
                    for c in range(CK - 1):
                        nc.tensor.matmul(
                            y_ps[:, lo:hi],
                            lhsT=attn_bf[c][:, n * 128:(n + 1) * 128],
                            rhs=wp_bf[c][:, lo:hi],
                            start=(c == 0),
                            stop=False,
                        )
            for n in grp:
                y_ps = y_tiles[n]
                for lo, hi in ((0, 512), (512, 768)):
                    nc.tensor.matmul(
                        y_ps[:, lo:hi],
                        lhsT=attn_bf[CK - 1][:, n * 128:(n + 1) * 128],
                        rhs=wp_bf[CK - 1][:, lo:hi],
                        start=False,
                        stop=True,
                    )
                y_sb = stage.tile([128, C], F32, name=f"y_sb{n}", tag="y", bufs=2)
                nc.vector.tensor_add(y_sb[:], y_ps[:], bias_bc[:])
                nc.sync.dma_start(out=out[n * 128:(n + 1) * 128, :], in_=y_sb[:])
        yps.release()


def build_graph():
    nc = bacc.Bacc("TRN2", target_bir_lowering=False, debug=False)
    xT = nc.declare_dram_parameter("xT", [C, N], BF16, isOutput=False)
    wqkvT = nc.declare_dram_parameter("wqkvT", [C, 3 * C], BF16, isOutput=False)
    wprojT = nc.declare_dram_parameter("wprojT", [C, C], BF16, isOutput=False)
    bproj = nc.declare_dram_parameter("bproj", [C], F32, isOutput=False)
    out = nc.declare_dram_parameter("out", [N, C], F32, isOutput=True)
    with tile.TileContext(nc) as tc:
        _emit(tc, xT.ap(), wqkvT.ap(), wprojT.ap(), bproj.ap(), out.ap())
    nc.compile()
    return nc


_GRAPH = None


def _get_graph():
    global _GRAPH
    if _GRAPH is None:
        _GRAPH = build_graph()
    return _GRAPH


def make_in_maps(x, W_qkv, W_proj, b_proj):
    import ml_dtypes

    x = np.asarray(x, dtype=np.float32)
    wqkvT = np.ascontiguousarray(
        np.asarray(W_qkv, dtype=np.float32).T.astype(ml_dtypes.bfloat16)
    )
    wprojT = np.ascontiguousarray(
        np.asarray(W_proj, dtype=np.float32).T.astype(ml_dtypes.bfloat16)
    )
    bp = np.ascontiguousarray(np.asarray(b_proj, dtype=np.float32))
    xT_all = np.ascontiguousarray(
        x.transpose(0, 2, 1).astype(ml_dtypes.bfloat16)
    )
    return [
        {"xT": xT_all[i], "wqkvT": wqkvT, "wprojT": wprojT, "bproj": bp}
        for i in range(B)
    ]


def run(x, W_qkv, W_proj, b_proj, trace=False):
    nc = _get_graph()
    in_maps = make_in_maps(x, W_qkv, W_proj, b_proj)
    res = run_bass_kernel_spmd(nc, in_maps, core_ids=list(range(B)), trace=trace)
    out = np.stack([res.results[i]["out"] for i in range(B)], axis=0)
    return out.astype(np.float32, copy=False), res


def kernel(x, W_qkv, W_proj, b_proj, H=None, W=None):
    out, _ = run(x, W_qkv, W_proj, b_proj)
    return out
